# revision 42
# baseline (speedup 1.0000x reference)
"""BloomAttention (B=1, S=2048, HID=4096, NH=32) on 8 Trainium2 NeuronCores.

Strategy (tensor-parallel over heads):
  - Each core owns 4 heads. w_qkv/b_qkv column-sharded; INV_NORM folded into
    the q columns on host; weights shipped transposed+bf16; hidden shipped
    PRE-TRANSPOSED (hiddenT [HID, S]) in bf16 so no on-device DMA-transpose
    is needed.
  - QKV: qT/kT [d, s] come from w.T @ hT matmuls; V is produced directly in
    NATURAL [s, d] layout by swapping the matmul operands (lhsT = hT tile,
    rhs = V weight columns), so no transpose / DRAM round-trip for V.
    V bias is folded in as a K=1 ones-row matmul at accumulation start.
  - Attention in transposed-scores layout: scoresT[sk, sq] = kT.T @ qT.
    The ALiBi bias slope*(sk-sq) (with the exact per-query shift) is applied
    as: (a) a K=1 rank-1 matmul adding slope*(-sq) (per-query-constant
    rounding cancels in softmax), (b) a per-partition bias slope*(tile_off +
    sk_within_tile) fused into the exp activation on ACT (free), and (c) a
    single shared [128,128] additive -1e9 mask strip on the causal diagonal.
    Diagonal score tiles are column-narrowed (fully-masked columns skipped).
    exp on ACT; P@V and the softmax denominator are matmuls over the sk
    partitions; normalization via ones-row broadcast matmul +
    reciprocal_approx_fast.
  - AllToAll (split in two, per head-pair) swaps head-shards for
    sequence-shards; dense is split into two k-half passes, one per
    AllToAll, so pass 0 overlaps attention of heads 2,3 and the second
    collective. Pass 1 accumulates into the DRAM output via CCE accum-DMA.

Note: assumes the alibi input is the standard Bloom form alibi[h, j] =
slope_h * j (slope read from alibi[:, 1]); the reference's setup_inputs
builds exactly that.
"""

import math
import os
import sys
import types
from contextlib import ExitStack

import numpy as np
import ml_dtypes

B, S, HID, NH, HD = 1, 2048, 4096, 32, 128
NCORES = 8
NH_LOC = NH // NCORES            # 4 heads per core
SROW = S // NCORES               # 256 output rows per core
INV_NORM = 1.0 / math.sqrt(HD)
KT = HID // HD                   # 32 contraction tiles
KC = 8                           # k tiles cached in SBUF (rest streamed)
KS = KT - KC                     # streamed k tiles (24)
NR = 19                          # distinct (sk-sq)/128 tile offsets

_CACHE = {}


def _ensure_axon_hooks():
    try:
        import antenv  # noqa: F401

        extra = "/opt/trn_rl_repo/antenv"
        if os.path.isdir(extra) and extra not in antenv.__path__:
            antenv.__path__.append(extra)
        import antenv.axon_hooks  # noqa: F401
    except Exception:
        if "antenv.axon_hooks" in sys.modules:
            return
        # Functional stand-in: the axon boot code (trn_boot.py) stores the
        # NTFF profiling hook here at jax init; bass_utils reads it back.
        m = types.ModuleType("antenv.axon_hooks")
        m._hook = None

        def _set(h, _m=m):
            _m._hook = h

        m.get_axon_ntff_profile_hook = lambda _m=m: _m._hook
        m.set_axon_ntff_profile_hook = _set
        sys.modules["antenv.axon_hooks"] = m
        try:
            from trn_agent_boot.trn_boot import _ntff_profile_via_ctypes

            so = "/opt/axon/libaxon_pjrt.so"
            if os.path.isfile(so):
                hook = _ntff_profile_via_ctypes(so)
                if hook is not None:
                    m._hook = hook
        except Exception:
            pass


_ensure_axon_hooks()


def _build_nc():
    import concourse.bass as bass  # noqa: F401
    import concourse.mybir as mybir
    from concourse import bacc, bass_isa, tile
    from concourse.tile import add_dep_helper

    BF = mybir.dt.bfloat16
    F32 = mybir.dt.float32
    Alu = mybir.AluOpType
    Act = mybir.ActivationFunctionType

    nc = bacc.Bacc(None, target_bir_lowering=False, num_devices=NCORES)
    with tile.TileContext(nc) as tc, ExitStack() as ctx:
        dram = ctx.enter_context(tc.tile_pool(name="dram", bufs=1, space="DRAM"))

        def din(name, shape, dt):
            return dram.tile(shape, dt, kind="ExternalInput", name=name,
                             uniquify=False)

        hiddenT = din("hiddenT", [HID, S], BF)
        # [g, p, kt, 512]: g0 = heads01 qk, g1 = v (all heads), g2 = h23 qk
        wqall = din("wqall", [3, HD, KT, 512], BF)
        bqk = din("bqk", [HD, 8], F32)          # per-feature q/k bias columns
        vbias = din("vbias", [HD, 512], F32)    # v bias bcast [4h x 128d]
        biasca = din("biasca", [HD, NH_LOC * NR], F32)  # slope*(off+a)
        negbbc = din("negbbc", [HD, 512], BF)   # -(0..511) bcast rows
        slmat = din("slmat", [HD, NH_LOC * HD], BF)  # slope_h/128 blocks
        maskst = din("maskst", [HD, HD], F32)   # 0 / -1e9 strip
        wdr = din("wdr", [NH_LOC, 8, HD, 8, 512], BF)
        bdense = din("bdense", [1, HID], F32)
        out = dram.tile([SROW, HID], F32, kind="ExternalOutput", name="out",
                        uniquify=False)
        a2a_in = [dram.tile([NCORES, HD, SROW], BF, name=f"a2a_in{p}")
                  for p in range(NH_LOC)]
        a2a_out = [dram.tile([NCORES, HD, SROW], BF, name=f"a2a_out{p}")
                   for p in range(NH_LOC)]

        # ---------- persistent SBUF ----------
        # consts go over SWDGE (gpsimd) so the HWDGE rings are free for the
        # startup-critical hidden/weight loads
        const = ctx.enter_context(tc.tile_pool(name="const", bufs=1))
        sb_bqk = const.tile([HD, 8], F32)
        nc.gpsimd.dma_start(out=sb_bqk[:], in_=bqk[:])
        sb_vbias = const.tile([HD, 512], F32)
        nc.gpsimd.dma_start(out=sb_vbias[:], in_=vbias[:])
        sb_bca = const.tile([HD, NH_LOC * NR], F32)
        nc.gpsimd.dma_start(out=sb_bca[:], in_=biasca[:])
        sb_negb = const.tile([HD, 512], BF)
        nc.gpsimd.dma_start(out=sb_negb[:], in_=negbbc[:])
        sb_slmat = const.tile([HD, NH_LOC * HD], BF)
        nc.gpsimd.dma_start(out=sb_slmat[:], in_=slmat[:])
        sb_mask = const.tile([HD, HD], F32)
        nc.gpsimd.dma_start(out=sb_mask[:], in_=maskst[:])
        ones_row = const.tile([1, HD], F32)
        nc.vector.memset(ones_row[:], 1.0)
        ones_mat = const.tile([HD, HD], BF)
        nc.vector.memset(ones_mat[:], 1.0)
        # pre-load the exp activation table set before it's first needed
        warm = const.tile([HD, 1], F32)
        nc.vector.memset(warm[:], 0.0)
        nc.scalar.activation(warm[:], warm[:], Act.Exp)

        persist = ctx.enter_context(tc.tile_pool(name="persist", bufs=1))
        qT = [persist.tile([HD, S], BF, name=f"qT{h}") for h in range(NH_LOC)]
        kTt = [persist.tile([HD, S], BF, name=f"kT{h}") for h in range(NH_LOC)]
        vnat = persist.tile([HD, NH_LOC, S], BF, name="vnat")

        # ---------- phase 1: QKV ----------
        with (
            tc.tile_pool(name="wqc", bufs=1) as wqc_pool,
            tc.tile_pool(name="wstream", bufs=4) as ws_pool,
            tc.tile_pool(name="hT", bufs=2) as hT_pool,
            tc.tile_pool(name="qkv_ps", bufs=1, space="PSUM") as qkv_ps,
        ):
            wq_c = wqc_pool.tile([HD, 3, KC, 512], BF)

            for sq in range(4):  # s-quarters of 512
                s0 = sq * 512
                hT_q = hT_pool.tile([HD, KT, 512], BF, name="hT_q")
                hsl = hiddenT[:, s0:s0 + 512].rearrange(
                    "(k p) s -> p k s", p=HD)
                if sq == 0:
                    # startup-critical: smallest prefix first — group-0
                    # cached weights + early hidden k-tiles, split across
                    # both HWDGE rings in consumption order
                    nc.sync.dma_start(out=wq_c[:, 0, :, :],
                                      in_=wqall[0, :, 0:KC, :])
                    nc.scalar.dma_start(out=hT_q[:, 0:8, :],
                                        in_=hsl[:, 0:8, :])
                    nc.sync.dma_start(out=hT_q[:, 8:16, :],
                                      in_=hsl[:, 8:16, :])
                    nc.scalar.dma_start(out=hT_q[:, 16:24, :],
                                        in_=hsl[:, 16:24, :])
                    nc.sync.dma_start(out=hT_q[:, 24:KT, :],
                                      in_=hsl[:, 24:KT, :])
                    nc.scalar.dma_start(out=wq_c[:, 1, :, :],
                                        in_=wqall[1, :, 0:KC, :])
                    nc.sync.dma_start(out=wq_c[:, 2, :, :],
                                      in_=wqall[2, :, 0:KC, :])
                else:
                    nc.sync.dma_start(out=hT_q[:, 0:KT // 2, :],
                                      in_=hsl[:, 0:KT // 2, :])
                    nc.scalar.dma_start(out=hT_q[:, KT // 2:KT, :],
                                        in_=hsl[:, KT // 2:KT, :])

                # stream chunks for this quarter, in consumption order:
                # g0 = heads01 qk, g1 = v, g2 = heads23 qk
                wsts = {}
                for g in range(3):
                    for half in range(2):
                        k0 = KC + half * (KS // 2)
                        wst = ws_pool.tile([HD, KS // 2, 512], BF, name="ws")
                        nc.sync.dma_start(
                            out=wst[:],
                            in_=wqall[g, :, k0:k0 + KS // 2, :])
                        wsts[(g, half)] = wst

                def wslice(g, kt):
                    if kt < KC:
                        return wq_c[:, g, kt, :]
                    half = (kt - KC) // (KS // 2)
                    return wsts[(g, half)][:, (kt - KC) % (KS // 2), :]

                # --- group 0: heads 0,1 q/k; group 1: V; group 2: h2,3 ---
                for g in range(3):
                    if g == 1:
                        # V natural: per 128-row subtile, all 4 heads
                        for ssub in range(4):
                            psv = qkv_ps.tile([HD, 512], F32, name="psv",
                                              bufs=2)
                            for kt in range(KT):
                                nc.tensor.matmul(
                                    psv[:],
                                    hT_q[:, kt,
                                         ssub * HD:(ssub + 1) * HD],
                                    wslice(1, kt),
                                    start=(kt == 0), stop=(kt == KT - 1))
                            sk0 = s0 + ssub * HD
                            nc.vector.tensor_tensor(
                                vnat[:, :, sk0:sk0 + HD],
                                psv[:].rearrange("p (h d) -> p h d",
                                                 h=NH_LOC),
                                sb_vbias[:].rearrange("p (h d) -> p h d",
                                                      h=NH_LOC),
                                Alu.add)
                    else:
                        fg = 0 if g == 0 else 1
                        psl = [qkv_ps.tile([HD, 512], F32, name="psqk",
                                           bufs=5) for _ in range(4)]
                        for kt in range(KT):
                            wsl = wslice(g, kt)
                            for i in range(4):
                                nc.tensor.matmul(
                                    psl[i][:],
                                    wsl[:, i * HD:(i + 1) * HD],
                                    hT_q[:, kt, :],
                                    start=(kt == 0), stop=(kt == KT - 1))
                        for i in range(4):
                            h = 2 * fg + i // 2
                            dest = (qT, kTt)[i % 2][h][:, s0:s0 + 512]
                            fcol = 4 * fg + i
                            nc.scalar.activation(
                                dest, psl[i][:], Act.Identity,
                                bias=sb_bqk[:, fcol:fcol + 1])

        # ---------- phase 2+3+4: attention, a2a, dense ----------
        with (
            tc.tile_pool(name="expp", bufs=2) as expp,
            tc.tile_pool(name="bcp", bufs=2) as bcp,
            tc.tile_pool(name="cxp", bufs=3) as cxp,
            tc.tile_pool(name="dns_sb", bufs=1) as dns_sb,
            tc.tile_pool(name="crecvp", bufs=2) as crecv_pool,
            tc.tile_pool(name="wd_pool", bufs=2) as wd_pool,
            tc.tile_pool(name="osb_pool", bufs=1) as osb_pool,
            tc.tile_pool(name="attn_ps", bufs=1, space="PSUM") as attn_ps,
            tc.tile_pool(name="sc_ps", bufs=3, space="PSUM") as sc_ps,
            tc.tile_pool(name="dns_ps", bufs=1, space="PSUM") as dns_ps,
        ):
            sb_bd = dns_sb.tile([1, HID], F32)
            nc.sync.dma_start(out=sb_bd[:], in_=bdense[:])

            def attention_head(h):
                marker = None
                for sqb in range(4):
                    q0 = sqb * 512
                    nsk = 4 * (sqb + 1)
                    ps_ctx = attn_ps.tile([HD, 512], F32, name="ps_ctx",
                                          bufs=2)
                    # column sums of the exp tiles, already broadcast to all
                    # partitions: accumulate ones[128,128].T @ ex on PE
                    ps_den = attn_ps.tile([HD, 512], F32, name="ps_den",
                                          bufs=2)
                    exb = expp.tile([HD, 16, 512], BF, name="exb")
                    c0s = {}

                    def flush(skt, first, last):
                        c0 = c0s[skt]
                        nc.tensor.matmul(
                            ps_ctx[:, c0:512],
                            vnat[:, h, skt * HD:(skt + 1) * HD],
                            exb[:, skt, c0:512], start=first, stop=last)
                        nc.tensor.matmul(
                            ps_den[:, c0:512], ones_mat[:],
                            exb[:, skt, c0:512], start=first, stop=last)

                    for skt in range(nsk):
                        i = skt - 4 * sqb    # >= 0 on the diagonal band
                        ri = i + 15
                        c0 = i * HD if i > 0 else 0
                        ps = sc_ps.tile([HD, 512], F32, name="ps_sc")
                        nc.tensor.matmul(
                            ps[:, c0:512],
                            kTt[h][:, skt * HD:(skt + 1) * HD],
                            qT[h][:, q0 + c0:q0 + 512],
                            start=True, stop=False)
                        nc.tensor.matmul(
                            ps[:, c0:512],
                            sb_slmat[:, h * HD:(h + 1) * HD],
                            sb_negb[:, c0:512],
                            start=False, stop=True)
                        if i >= 0:
                            nc.vector.tensor_tensor(
                                ps[:, c0:c0 + HD], ps[:, c0:c0 + HD],
                                sb_mask[:], Alu.add)
                        nc.scalar.activation(
                            exb[:, skt, c0:512], ps[:, c0:512], Act.Exp,
                            bias=sb_bca[:, h * NR + ri:h * NR + ri + 1])
                        c0s[skt] = c0
                        if skt >= 2:
                            flush(skt - 2, skt - 2 == 0, False)
                    for skt in (nsk - 2, nsk - 1):
                        flush(skt, skt == 0, skt == nsk - 1)

                    rec_bc = bcp.tile([HD, 512], F32, name="rec_bc")
                    nc.vector.reciprocal_approx_fast(out=rec_bc[:],
                                                     in_=ps_den[:])
                    cxc = cxp.tile([HD, 512], BF, name="cxc")
                    mul_inst = nc.vector.tensor_tensor(
                        cxc[:], ps_ctx[:], rec_bc[:], Alu.mult)
                    if sqb == 3:
                        marker = mul_inst
                    for jj in range(2):
                        j = 2 * sqb + jj
                        nc.scalar.dma_start(
                            out=a2a_in[h][j],
                            in_=cxc[:, jj * SROW:(jj + 1) * SROW])
                return marker

            osbs = {}

            def dense_pass(p, order_after=None):
                crecv = crecv_pool.tile([HD, NCORES, SROW], BF,
                                        name="crecv")
                for i in range(NCORES):
                    cr = nc.sync.dma_start(out=crecv[:, i, :],
                                           in_=a2a_out[p][i])
                    if order_after is not None:
                        add_dep_helper(cr.ins, order_after.ins, sync=False,
                                       reason="pass after next-head mid")
                for ot in range(8):
                    o0 = ot * 512
                    wd = wd_pool.tile([HD, 8, 512], BF, name="wd")
                    nc.sync.dma_start(out=wd[:], in_=wdr[p, ot])
                    for st in range(2):
                        psd = dns_ps.tile([HD, 512], F32, name="psd")
                        if p == 0:
                            nc.tensor.matmul(
                                psd[:], ones_row[:], sb_bd[:, o0:o0 + 512],
                                start=True, stop=False)
                        for ft in range(8):
                            nc.tensor.matmul(
                                psd[:],
                                crecv[:, ft, st * HD:(st + 1) * HD],
                                wd[:, ft, :],
                                start=(p > 0 and ft == 0), stop=(ft == 7))
                        if p == 0:
                            osb = osb_pool.tile([HD, 512], F32,
                                                name=f"osb{ot}_{st}")
                            osbs[(ot, st)] = osb
                            nc.vector.tensor_copy(osb[:], psd[:])
                        else:
                            osb = osbs[(ot, st)]
                            nc.vector.tensor_tensor(
                                osb[:], osb[:], psd[:], Alu.add)
                        if p == NH_LOC - 1:
                            nc.sync.dma_start(
                                out=out[st * HD:(st + 1) * HD, o0:o0 + 512],
                                in_=osb[:])

            markers = []
            for h in range(NH_LOC):
                markers.append(attention_head(h))
                nc.gpsimd.collective_compute(
                    "AllToAll", Alu.bypass,
                    replica_groups=[list(range(NCORES))],
                    ins=[a2a_in[h][:]], outs=[a2a_out[h][:]])
            # order pass p's start after the middle of head p+1's attention
            # so its collective-gated matmuls don't clog the PE FIFO while
            # the (slow) collective is still in flight
            for p in range(NH_LOC):
                after = markers[p + 1] if p + 1 < NH_LOC else None
                dense_pass(p, order_after=after)
    nc.compile()
    return nc


def _prep_shards(hidden_states, alibi, w_qkv, b_qkv, w_dense, b_dense):
    bf16 = ml_dtypes.bfloat16
    hid = np.asarray(hidden_states, dtype=np.float32).reshape(S, HID)
    hiddenT = np.ascontiguousarray(hid.T).astype(bf16)      # [HID, S]
    al = np.asarray(alibi, dtype=np.float32).reshape(NH, S)
    w = np.asarray(w_qkv, dtype=np.float32)
    b = np.asarray(b_qkv, dtype=np.float32)
    wd = np.asarray(w_dense, dtype=np.float32)
    bd = np.asarray(b_dense, dtype=np.float32)

    # fold INV_NORM into the q projections
    scale = np.ones(3 * HID, np.float32)
    for h in range(NH):
        scale[h * 3 * HD:(h * 3 * HD) + HD] = INV_NORM
    wT = np.ascontiguousarray((w * scale[:, None]).T)      # [HID, 3*HID]
    bs = b * scale

    # dense weights, transposed then per-pass/ot tiled:
    # wdr[p, ot, 128, 8, 512]; k-tile ft=i <-> global head 4i+p
    wdT = np.ascontiguousarray(wd.T).astype(np.float32)    # [HID(f), HID(o)]
    wdr = np.empty((NH_LOC, 8, HD, NCORES, 512), np.float32)
    for p in range(NH_LOC):
        for i in range(NCORES):
            g = 4 * i + p
            blk = wdT[g * HD:(g + 1) * HD]                 # [128, 4096]
            wdr[p, :, :, i, :] = blk.reshape(HD, 8, 512).transpose(1, 0, 2)
    wdr = wdr.astype(bf16)
    bdr = np.ascontiguousarray(bd.reshape(1, HID))

    # mask strip: 0 where a <= b, -1e9 where a > b (future key)
    a = np.arange(HD)[:, None]
    bcol = np.arange(HD)[None, :]
    maskst = np.where(a <= bcol, 0.0, -1.0e9).astype(np.float32)
    negbbc = np.ascontiguousarray(np.broadcast_to(
        -np.arange(512, dtype=np.float32).reshape(1, 512),
        (HD, 512))).astype(bf16)

    in_maps = []
    for c in range(NCORES):
        heads = list(range(c * NH_LOC, (c + 1) * NH_LOC))
        slopes = al[heads, 1]                              # [4]
        # wqall: [3, 128, KT, 512]; g0 = [q0 k0 q1 k1], g1 = v, g2 = h2,h3
        cols = []
        qkcols = []
        for h in range(NH_LOC):
            base = (heads[h]) * 3 * HD
            qkcols += [list(range(base, base + HD)),
                       list(range(base + HD, base + 2 * HD))]
        vcols = []
        for h in range(NH_LOC):
            base = heads[h] * 3 * HD + 2 * HD
            vcols += list(range(base, base + HD))
        for i in range(4):
            cols += qkcols[i]
        cols += vcols
        for i in range(4):
            cols += qkcols[4 + i]
        wq = wT[:, cols]                                   # [HID, 1536]
        wqall = np.ascontiguousarray(
            wq.reshape(KT, HD, 3, 512).transpose(2, 1, 0, 3)).astype(bf16)
        bqk = np.stack(
            [bs[qkcols[f]] for f in range(8)], axis=1)     # [128, 8]
        vbias = np.broadcast_to(bs[vcols].reshape(1, 512), (HD, 512))
        biasca = np.empty((HD, NH_LOC * NR), np.float32)
        for h in range(NH_LOC):
            for ri in range(NR):
                biasca[:, h * NR + ri] = slopes[h] * (
                    (ri - 15) * HD + np.arange(HD, dtype=np.float32))
        slmat = np.repeat(slopes / HD, HD).reshape(1, NH_LOC * HD)
        slmat = np.broadcast_to(slmat, (HD, NH_LOC * HD))
        in_maps.append({
            "hiddenT": hiddenT,
            "wqall": wqall,
            "bqk": np.ascontiguousarray(bqk.astype(np.float32)),
            "vbias": np.ascontiguousarray(vbias.astype(np.float32)),
            "biasca": np.ascontiguousarray(biasca),
            "negbbc": negbbc,
            "slmat": np.ascontiguousarray(
                slmat.astype(np.float32)).astype(bf16),
            "maskst": maskst,
            "wdr": wdr,
            "bdense": bdr,
        })
    return in_maps


def kernel(hidden_states, alibi, w_qkv, b_qkv, w_dense, b_dense):
    _ensure_axon_hooks()
    from concourse import bass_utils

    if "nc" not in _CACHE:
        _CACHE["nc"] = _build_nc()
    nc = _CACHE["nc"]
    in_maps = _prep_shards(hidden_states, alibi, w_qkv, b_qkv,
                           w_dense, b_dense)
    trace = bool(os.environ.get("BLOOM_TRACE"))
    res = bass_utils.run_bass_kernel_spmd(
        nc, in_maps, core_ids=list(range(NCORES)), trace=trace)
    kernel._last_results = res
    kernel._last_exec_ns = res.exec_time_ns
    outp = np.concatenate([res.results[c]["out"] for c in range(NCORES)],
                          axis=0)
    return outp.reshape(B, S, HID).astype(np.float32)


# revision 44
# speedup vs baseline: 1.0888x; 1.0888x over previous
"""BloomAttention (B=1, S=2048, HID=4096, NH=32) on 8 Trainium2 NeuronCores.

Strategy (tensor-parallel over heads):
  - Each core owns 4 heads. w_qkv/b_qkv column-sharded; INV_NORM folded into
    the q columns on host; weights shipped transposed+bf16; hidden shipped
    PRE-TRANSPOSED (hiddenT [HID, S]) in bf16 so no on-device DMA-transpose
    is needed.
  - QKV: qT/kT [d, s] come from w.T @ hT matmuls; V is produced directly in
    NATURAL [s, d] layout by swapping the matmul operands (lhsT = hT tile,
    rhs = V weight columns), so no transpose / DRAM round-trip for V.
    V bias is folded in as a K=1 ones-row matmul at accumulation start.
  - Attention in transposed-scores layout: scoresT[sk, sq] = kT.T @ qT.
    The ALiBi bias slope*(sk-sq) (with the exact per-query shift) is applied
    as: (a) a K=1 rank-1 matmul adding slope*(-sq) (per-query-constant
    rounding cancels in softmax), (b) a per-partition bias slope*(tile_off +
    sk_within_tile) fused into the exp activation on ACT (free), and (c) a
    single shared [128,128] additive -1e9 mask strip on the causal diagonal.
    Diagonal score tiles are column-narrowed (fully-masked columns skipped).
    exp on ACT; P@V and the softmax denominator are matmuls over the sk
    partitions; normalization via ones-row broadcast matmul +
    reciprocal_approx_fast.
  - AllToAll (split in two, per head-pair) swaps head-shards for
    sequence-shards; dense is split into two k-half passes, one per
    AllToAll, so pass 0 overlaps attention of heads 2,3 and the second
    collective. Pass 1 accumulates into the DRAM output via CCE accum-DMA.

Note: assumes the alibi input is the standard Bloom form alibi[h, j] =
slope_h * j (slope read from alibi[:, 1]); the reference's setup_inputs
builds exactly that.
"""

import math
import os
import sys
import types
from contextlib import ExitStack

import numpy as np
import ml_dtypes

B, S, HID, NH, HD = 1, 2048, 4096, 32, 128
NCORES = 8
NH_LOC = NH // NCORES            # 4 heads per core
SROW = S // NCORES               # 256 output rows per core
INV_NORM = 1.0 / math.sqrt(HD)
KT = HID // HD                   # 32 contraction tiles
KC = 8                           # k tiles cached in SBUF (rest streamed)
KS = KT - KC                     # streamed k tiles (24)
NR = 19                          # distinct (sk-sq)/128 tile offsets

_CACHE = {}


def _ensure_axon_hooks():
    try:
        import antenv  # noqa: F401

        extra = "/opt/trn_rl_repo/antenv"
        if os.path.isdir(extra) and extra not in antenv.__path__:
            antenv.__path__.append(extra)
        import antenv.axon_hooks  # noqa: F401
    except Exception:
        if "antenv.axon_hooks" in sys.modules:
            return
        # Functional stand-in: the axon boot code (trn_boot.py) stores the
        # NTFF profiling hook here at jax init; bass_utils reads it back.
        m = types.ModuleType("antenv.axon_hooks")
        m._hook = None

        def _set(h, _m=m):
            _m._hook = h

        m.get_axon_ntff_profile_hook = lambda _m=m: _m._hook
        m.set_axon_ntff_profile_hook = _set
        sys.modules["antenv.axon_hooks"] = m
        try:
            from trn_agent_boot.trn_boot import _ntff_profile_via_ctypes

            so = "/opt/axon/libaxon_pjrt.so"
            if os.path.isfile(so):
                hook = _ntff_profile_via_ctypes(so)
                if hook is not None:
                    m._hook = hook
        except Exception:
            pass


_ensure_axon_hooks()


def _build_nc():
    import concourse.bass as bass  # noqa: F401
    import concourse.mybir as mybir
    from concourse import bacc, bass_isa, tile
    from concourse.tile import add_dep_helper

    BF = mybir.dt.bfloat16
    F32 = mybir.dt.float32
    Alu = mybir.AluOpType
    Act = mybir.ActivationFunctionType

    nc = bacc.Bacc(None, target_bir_lowering=False, num_devices=NCORES)
    with tile.TileContext(nc) as tc, ExitStack() as ctx:
        dram = ctx.enter_context(tc.tile_pool(name="dram", bufs=1, space="DRAM"))

        def din(name, shape, dt):
            return dram.tile(shape, dt, kind="ExternalInput", name=name,
                             uniquify=False)

        hiddenT = din("hiddenT", [HID, S], BF)
        # [g, p, kt, 512]: g0 = heads01 qk, g1 = v (all heads), g2 = h23 qk
        wqall = din("wqall", [3, HD, KT, 512], BF)
        bqk = din("bqk", [HD, 8], F32)          # per-feature q/k bias columns
        vbias = din("vbias", [HD, 512], F32)    # v bias bcast [4h x 128d]
        biasca = din("biasca", [HD, NH_LOC * NR], F32)  # slope*(off+a)
        negbbc = din("negbbc", [HD, 512], BF)   # -(0..511) bcast rows
        slmat = din("slmat", [HD, NH_LOC * HD], BF)  # slope_h/128 blocks
        maskst = din("maskst", [HD, HD], F32)   # 0 / -1e9 strip
        wdr = din("wdr", [NH_LOC, 8, HD, 8, 512], BF)
        bdense = din("bdense", [1, HID], F32)
        out = dram.tile([SROW, HID], F32, kind="ExternalOutput", name="out",
                        uniquify=False)
        a2a_in = [dram.tile([NCORES, HD, SROW], BF, name=f"a2a_in{p}")
                  for p in range(NH_LOC)]
        a2a_out = [dram.tile([NCORES, HD, SROW], BF, name=f"a2a_out{p}")
                   for p in range(NH_LOC)]

        # ---------- persistent SBUF ----------
        # consts go over SWDGE (gpsimd) so the HWDGE rings are free for the
        # startup-critical hidden/weight loads
        const = ctx.enter_context(tc.tile_pool(name="const", bufs=1))
        sb_bqk = const.tile([HD, 8], F32)
        nc.gpsimd.dma_start(out=sb_bqk[:], in_=bqk[:])
        sb_vbias = const.tile([HD, 512], F32)
        nc.gpsimd.dma_start(out=sb_vbias[:], in_=vbias[:])
        sb_bca = const.tile([HD, NH_LOC * NR], F32)
        nc.gpsimd.dma_start(out=sb_bca[:], in_=biasca[:])
        sb_negb = const.tile([HD, 512], BF)
        nc.gpsimd.dma_start(out=sb_negb[:], in_=negbbc[:])
        sb_slmat = const.tile([HD, NH_LOC * HD], BF)
        nc.gpsimd.dma_start(out=sb_slmat[:], in_=slmat[:])
        sb_mask = const.tile([HD, HD], F32)
        nc.gpsimd.dma_start(out=sb_mask[:], in_=maskst[:])
        ones_row = const.tile([1, HD], F32)
        nc.vector.memset(ones_row[:], 1.0)
        ones_mat = const.tile([HD, HD], BF)
        nc.vector.memset(ones_mat[:], 1.0)
        # pre-load the exp activation table set before it's first needed
        warm = const.tile([HD, 1], F32)
        nc.vector.memset(warm[:], 0.0)
        nc.scalar.activation(warm[:], warm[:], Act.Exp)

        persist = ctx.enter_context(tc.tile_pool(name="persist", bufs=1))
        qT = [persist.tile([HD, S], BF, name=f"qT{h}") for h in range(NH_LOC)]
        kTt = [persist.tile([HD, S], BF, name=f"kT{h}") for h in range(NH_LOC)]
        vnat = persist.tile([HD, NH_LOC, S], BF, name="vnat")

        # ---------- phase 1: QKV ----------
        with (
            tc.tile_pool(name="wqc", bufs=1) as wqc_pool,
            tc.tile_pool(name="wstream", bufs=4) as ws_pool,
            tc.tile_pool(name="hT", bufs=2) as hT_pool,
            tc.tile_pool(name="qkv_ps", bufs=1, space="PSUM") as qkv_ps,
        ):
            wq_c = wqc_pool.tile([HD, 3, KC, 512], BF)

            for sq in range(4):  # s-quarters of 512
                s0 = sq * 512
                hT_q = hT_pool.tile([HD, KT, 512], BF, name="hT_q")
                hsl = hiddenT[:, s0:s0 + 512].rearrange(
                    "(k p) s -> p k s", p=HD)
                if sq == 0:
                    # startup-critical: smallest prefix first — group-0
                    # cached weights + early hidden k-tiles, split across
                    # both HWDGE rings in consumption order
                    nc.sync.dma_start(out=wq_c[:, 0, :, :],
                                      in_=wqall[0, :, 0:KC, :])
                    nc.scalar.dma_start(out=hT_q[:, 0:8, :],
                                        in_=hsl[:, 0:8, :])
                    nc.sync.dma_start(out=hT_q[:, 8:16, :],
                                      in_=hsl[:, 8:16, :])
                    nc.scalar.dma_start(out=hT_q[:, 16:24, :],
                                        in_=hsl[:, 16:24, :])
                    nc.sync.dma_start(out=hT_q[:, 24:KT, :],
                                      in_=hsl[:, 24:KT, :])
                    nc.scalar.dma_start(out=wq_c[:, 1, :, :],
                                        in_=wqall[1, :, 0:KC, :])
                    nc.sync.dma_start(out=wq_c[:, 2, :, :],
                                      in_=wqall[2, :, 0:KC, :])
                else:
                    nc.sync.dma_start(out=hT_q[:, 0:KT // 2, :],
                                      in_=hsl[:, 0:KT // 2, :])
                    nc.scalar.dma_start(out=hT_q[:, KT // 2:KT, :],
                                        in_=hsl[:, KT // 2:KT, :])

                # stream chunks for this quarter, in consumption order:
                # g0 = heads01 qk, g1 = v, g2 = heads23 qk
                wsts = {}
                for g in range(3):
                    for half in range(2):
                        k0 = KC + half * (KS // 2)
                        wst = ws_pool.tile([HD, KS // 2, 512], BF, name="ws")
                        nc.sync.dma_start(
                            out=wst[:],
                            in_=wqall[g, :, k0:k0 + KS // 2, :])
                        wsts[(g, half)] = wst

                def wslice(g, kt):
                    if kt < KC:
                        return wq_c[:, g, kt, :]
                    half = (kt - KC) // (KS // 2)
                    return wsts[(g, half)][:, (kt - KC) % (KS // 2), :]

                # --- group 0: heads 0,1 q/k; group 1: V; group 2: h2,3 ---
                for g in range(3):
                    if g == 1:
                        # V natural: per 128-row subtile, all 4 heads
                        for ssub in range(4):
                            psv = qkv_ps.tile([HD, 512], F32, name="psv",
                                              bufs=2)
                            for kt in range(KT):
                                nc.tensor.matmul(
                                    psv[:],
                                    hT_q[:, kt,
                                         ssub * HD:(ssub + 1) * HD],
                                    wslice(1, kt),
                                    start=(kt == 0), stop=(kt == KT - 1))
                            sk0 = s0 + ssub * HD
                            nc.vector.tensor_tensor(
                                vnat[:, :, sk0:sk0 + HD],
                                psv[:].rearrange("p (h d) -> p h d",
                                                 h=NH_LOC),
                                sb_vbias[:].rearrange("p (h d) -> p h d",
                                                      h=NH_LOC),
                                Alu.add)
                    else:
                        fg = 0 if g == 0 else 1
                        psl = [qkv_ps.tile([HD, 512], F32, name="psqk",
                                           bufs=5) for _ in range(4)]
                        for kt in range(KT):
                            wsl = wslice(g, kt)
                            for i in range(4):
                                nc.tensor.matmul(
                                    psl[i][:],
                                    wsl[:, i * HD:(i + 1) * HD],
                                    hT_q[:, kt, :],
                                    start=(kt == 0), stop=(kt == KT - 1))
                        for i in range(4):
                            h = 2 * fg + i // 2
                            dest = (qT, kTt)[i % 2][h][:, s0:s0 + 512]
                            fcol = 4 * fg + i
                            nc.scalar.activation(
                                dest, psl[i][:], Act.Identity,
                                bias=sb_bqk[:, fcol:fcol + 1])

        # ---------- phase 2+3+4: attention, a2a, dense ----------
        with (
            tc.tile_pool(name="expp", bufs=2) as expp,
            tc.tile_pool(name="bcp", bufs=2) as bcp,
            tc.tile_pool(name="cxp", bufs=3) as cxp,
            tc.tile_pool(name="dns_sb", bufs=1) as dns_sb,
            tc.tile_pool(name="crecvp", bufs=2) as crecv_pool,
            tc.tile_pool(name="wd_pool", bufs=2) as wd_pool,
            tc.tile_pool(name="osb_pool", bufs=1) as osb_pool,
            tc.tile_pool(name="attn_ps", bufs=1, space="PSUM") as attn_ps,
            tc.tile_pool(name="sc_ps", bufs=3, space="PSUM") as sc_ps,
            tc.tile_pool(name="dns_ps", bufs=2, space="PSUM") as dns_ps,
        ):
            sb_bd = dns_sb.tile([1, HID], F32)
            nc.sync.dma_start(out=sb_bd[:], in_=bdense[:])

            def attention_head(h):
                marker = None
                for sqb in range(4):
                    q0 = sqb * 512
                    nsk = 4 * (sqb + 1)
                    ps_ctx = attn_ps.tile([HD, 512], F32, name="ps_ctx",
                                          bufs=2)
                    # column sums of the exp tiles, already broadcast to all
                    # partitions: accumulate ones[128,128].T @ ex on PE
                    ps_den = attn_ps.tile([HD, 512], F32, name="ps_den",
                                          bufs=1)
                    exb = expp.tile([HD, 16, 512], BF, name="exb")
                    c0s = {}

                    def flush(skt, first, last):
                        c0 = c0s[skt]
                        nc.tensor.matmul(
                            ps_ctx[:, c0:512],
                            vnat[:, h, skt * HD:(skt + 1) * HD],
                            exb[:, skt, c0:512], start=first, stop=last)
                        nc.tensor.matmul(
                            ps_den[:, c0:512], ones_mat[:],
                            exb[:, skt, c0:512], start=first, stop=last)

                    for skt in range(nsk):
                        i = skt - 4 * sqb    # >= 0 on the diagonal band
                        ri = i + 15
                        c0 = i * HD if i > 0 else 0
                        ps = sc_ps.tile([HD, 512], F32, name="ps_sc")
                        nc.tensor.matmul(
                            ps[:, c0:512],
                            kTt[h][:, skt * HD:(skt + 1) * HD],
                            qT[h][:, q0 + c0:q0 + 512],
                            start=True, stop=False)
                        nc.tensor.matmul(
                            ps[:, c0:512],
                            sb_slmat[:, h * HD:(h + 1) * HD],
                            sb_negb[:, c0:512],
                            start=False, stop=True)
                        if i >= 0:
                            nc.vector.tensor_tensor(
                                ps[:, c0:c0 + HD], ps[:, c0:c0 + HD],
                                sb_mask[:], Alu.add)
                        nc.scalar.activation(
                            exb[:, skt, c0:512], ps[:, c0:512], Act.Exp,
                            bias=sb_bca[:, h * NR + ri:h * NR + ri + 1])
                        c0s[skt] = c0
                        if skt >= 2:
                            flush(skt - 2, skt - 2 == 0, False)
                    for skt in (nsk - 2, nsk - 1):
                        flush(skt, skt == 0, skt == nsk - 1)

                    rec_bc = bcp.tile([HD, 512], F32, name="rec_bc")
                    nc.vector.reciprocal_approx_fast(out=rec_bc[:],
                                                     in_=ps_den[:])
                    cxc = cxp.tile([HD, 512], BF, name="cxc")
                    mul_inst = nc.vector.tensor_tensor(
                        cxc[:], ps_ctx[:], rec_bc[:], Alu.mult)
                    if sqb == 3:
                        marker = mul_inst
                    for jj in range(2):
                        j = 2 * sqb + jj
                        nc.scalar.dma_start(
                            out=a2a_in[h][j],
                            in_=cxc[:, jj * SROW:(jj + 1) * SROW])
                return marker

            osbs = {}

            def dense_pass(p, order_after=None):
                crecv = crecv_pool.tile([HD, NCORES, SROW], BF,
                                        name="crecv")
                for i in range(NCORES):
                    cr = nc.sync.dma_start(out=crecv[:, i, :],
                                           in_=a2a_out[p][i])
                    if order_after is not None:
                        add_dep_helper(cr.ins, order_after.ins, sync=False,
                                       reason="pass after next-head mid")
                for ot in range(8):
                    o0 = ot * 512
                    wd = wd_pool.tile([HD, 8, 512], BF, name="wd")
                    nc.sync.dma_start(out=wd[:], in_=wdr[p, ot])
                    for st in range(2):
                        psd = dns_ps.tile([HD, 512], F32, name="psd")
                        if p == 0:
                            nc.tensor.matmul(
                                psd[:], ones_row[:], sb_bd[:, o0:o0 + 512],
                                start=True, stop=False)
                        for ft in range(8):
                            nc.tensor.matmul(
                                psd[:],
                                crecv[:, ft, st * HD:(st + 1) * HD],
                                wd[:, ft, :],
                                start=(p > 0 and ft == 0), stop=(ft == 7))
                        if p == 0:
                            osb = osb_pool.tile([HD, 512], F32,
                                                name=f"osb{ot}_{st}")
                            osbs[(ot, st)] = osb
                            nc.vector.tensor_copy(osb[:], psd[:])
                        else:
                            osb = osbs[(ot, st)]
                            nc.vector.tensor_tensor(
                                osb[:], osb[:], psd[:], Alu.add)
                        if p == NH_LOC - 1:
                            nc.sync.dma_start(
                                out=out[st * HD:(st + 1) * HD, o0:o0 + 512],
                                in_=osb[:])

            markers = []
            for h in range(NH_LOC):
                markers.append(attention_head(h))
                nc.gpsimd.collective_compute(
                    "AllToAll", Alu.bypass,
                    replica_groups=[list(range(NCORES))],
                    ins=[a2a_in[h][:]], outs=[a2a_out[h][:]])
            # order pass p's start after the middle of head p+1's attention
            # so its collective-gated matmuls don't clog the PE FIFO while
            # the (slow) collective is still in flight
            for p in range(NH_LOC):
                after = markers[p + 1] if p + 1 < NH_LOC else None
                dense_pass(p, order_after=after)
    nc.compile()
    return nc


def _prep_shards(hidden_states, alibi, w_qkv, b_qkv, w_dense, b_dense):
    bf16 = ml_dtypes.bfloat16
    hid = np.asarray(hidden_states, dtype=np.float32).reshape(S, HID)
    hiddenT = np.ascontiguousarray(hid.T).astype(bf16)      # [HID, S]
    al = np.asarray(alibi, dtype=np.float32).reshape(NH, S)
    w = np.asarray(w_qkv, dtype=np.float32)
    b = np.asarray(b_qkv, dtype=np.float32)
    wd = np.asarray(w_dense, dtype=np.float32)
    bd = np.asarray(b_dense, dtype=np.float32)

    # fold INV_NORM into the q projections
    scale = np.ones(3 * HID, np.float32)
    for h in range(NH):
        scale[h * 3 * HD:(h * 3 * HD) + HD] = INV_NORM
    wT = np.ascontiguousarray((w * scale[:, None]).T)      # [HID, 3*HID]
    bs = b * scale

    # dense weights, transposed then per-pass/ot tiled:
    # wdr[p, ot, 128, 8, 512]; k-tile ft=i <-> global head 4i+p
    wdT = np.ascontiguousarray(wd.T).astype(np.float32)    # [HID(f), HID(o)]
    wdr = np.empty((NH_LOC, 8, HD, NCORES, 512), np.float32)
    for p in range(NH_LOC):
        for i in range(NCORES):
            g = 4 * i + p
            blk = wdT[g * HD:(g + 1) * HD]                 # [128, 4096]
            wdr[p, :, :, i, :] = blk.reshape(HD, 8, 512).transpose(1, 0, 2)
    wdr = wdr.astype(bf16)
    bdr = np.ascontiguousarray(bd.reshape(1, HID))

    # mask strip: 0 where a <= b, -1e9 where a > b (future key)
    a = np.arange(HD)[:, None]
    bcol = np.arange(HD)[None, :]
    maskst = np.where(a <= bcol, 0.0, -1.0e9).astype(np.float32)
    negbbc = np.ascontiguousarray(np.broadcast_to(
        -np.arange(512, dtype=np.float32).reshape(1, 512),
        (HD, 512))).astype(bf16)

    in_maps = []
    for c in range(NCORES):
        heads = list(range(c * NH_LOC, (c + 1) * NH_LOC))
        slopes = al[heads, 1]                              # [4]
        # wqall: [3, 128, KT, 512]; g0 = [q0 k0 q1 k1], g1 = v, g2 = h2,h3
        cols = []
        qkcols = []
        for h in range(NH_LOC):
            base = (heads[h]) * 3 * HD
            qkcols += [list(range(base, base + HD)),
                       list(range(base + HD, base + 2 * HD))]
        vcols = []
        for h in range(NH_LOC):
            base = heads[h] * 3 * HD + 2 * HD
            vcols += list(range(base, base + HD))
        for i in range(4):
            cols += qkcols[i]
        cols += vcols
        for i in range(4):
            cols += qkcols[4 + i]
        wq = wT[:, cols]                                   # [HID, 1536]
        wqall = np.ascontiguousarray(
            wq.reshape(KT, HD, 3, 512).transpose(2, 1, 0, 3)).astype(bf16)
        bqk = np.stack(
            [bs[qkcols[f]] for f in range(8)], axis=1)     # [128, 8]
        vbias = np.broadcast_to(bs[vcols].reshape(1, 512), (HD, 512))
        biasca = np.empty((HD, NH_LOC * NR), np.float32)
        for h in range(NH_LOC):
            for ri in range(NR):
                biasca[:, h * NR + ri] = slopes[h] * (
                    (ri - 15) * HD + np.arange(HD, dtype=np.float32))
        slmat = np.repeat(slopes / HD, HD).reshape(1, NH_LOC * HD)
        slmat = np.broadcast_to(slmat, (HD, NH_LOC * HD))
        in_maps.append({
            "hiddenT": hiddenT,
            "wqall": wqall,
            "bqk": np.ascontiguousarray(bqk.astype(np.float32)),
            "vbias": np.ascontiguousarray(vbias.astype(np.float32)),
            "biasca": np.ascontiguousarray(biasca),
            "negbbc": negbbc,
            "slmat": np.ascontiguousarray(
                slmat.astype(np.float32)).astype(bf16),
            "maskst": maskst,
            "wdr": wdr,
            "bdense": bdr,
        })
    return in_maps


def kernel(hidden_states, alibi, w_qkv, b_qkv, w_dense, b_dense):
    _ensure_axon_hooks()
    from concourse import bass_utils

    if "nc" not in _CACHE:
        _CACHE["nc"] = _build_nc()
    nc = _CACHE["nc"]
    in_maps = _prep_shards(hidden_states, alibi, w_qkv, b_qkv,
                           w_dense, b_dense)
    trace = bool(os.environ.get("BLOOM_TRACE"))
    res = bass_utils.run_bass_kernel_spmd(
        nc, in_maps, core_ids=list(range(NCORES)), trace=trace)
    kernel._last_results = res
    kernel._last_exec_ns = res.exec_time_ns
    outp = np.concatenate([res.results[c]["out"] for c in range(NCORES)],
                          axis=0)
    return outp.reshape(B, S, HID).astype(np.float32)
